# revision 31
# baseline (speedup 1.0000x reference)
"""Trainium2 Bass kernel for nn_Block_34711925686730 (dense_transformer).

Pipeline per image (data-parallel over batch, 4 images / NeuronCore):
  LN(channels) -> iterative KL-NNMF grouped conv (25 iters) -> residual
  -> LN(channels) -> MLP (gelu) -> residual.

Fast NNMF loop (validated numerically against the jax reference in fp8):
  - h and nu stored fp8e4 (h scaled by HS=256, weights by WS=128; scales
    cancel through the per-iteration column normalization).
  - Grouped 3x3 convs run as fp8 DoubleRow matmuls: taps paired two per
    matmul (4 real pairs + [tap8, zero-tap]), 0.5 cycles/col -- 4x the
    bf16 column rate.  Tap pairs are built as raw strided APs over the
    padded [30,30] image with the pair axis stride = tap offset delta.
  - nu = x/recon computed in the log domain: ACT Ln(recon_psum),
    one bf16 DVE subtract vs precomputed ln(x), ACT Exp -> fp8.  nu is
    refreshed every 2nd iteration (numerically validated).
  - u = h*conv stays bf16; per-iter colsum normalize has HS folded into
    the ones weights; the last iteration uses exact ones and writes h in
    bf16 for the residual.
LN stat sums and fc1 run in fp32r; fc2 in bf16.
"""

import os
import numpy as np

DIM = 384
HEADS = 6
ITERS = int(os.environ.get("K_ITERS", "25"))
NB = int(os.environ.get("K_NB", "4"))  # images per core
MLP_HID = 4 * DIM
EPS = 1e-6
H = W = 28
NCORES = 8
NBLK = 3          # channel blocks of 128
PW = 30           # padded width
PLEN = 900        # padded spatial length (30*30)
R0 = 30           # stats/MLP range start (even, = (1,0))
RL = 840          # stats/MLP columns [30, 870)
NJ = MLP_HID // 128  # 12
NU_EVERY = int(os.environ.get("K_NU_EVERY", "2"))
HS = 256.0        # h scale (fp8 range)
WS = 128.0        # weight scale (fp8 range)
SCL = HS * WS
W1S = 64.0        # fc1 weight scale (fp8)
W2S = 16.0        # fc2 weight scale (fp8)
# GPSIMD cannot read PSUM (BIR verifier), so u-mul must stay on DVE.
POOL_HMUL = int(os.environ.get("K_POOL_HMUL", "2"))   # h-mul blocks on Pool

_cache = {}


def _round_tf32(a):
    bits = np.ascontiguousarray(a, dtype=np.float32).view(np.uint32)
    r = bits + np.uint32(0x0FFF) + ((bits >> np.uint32(13)) & np.uint32(1))
    return (r & np.uint32(0xFFFFE000)).view(np.float32).copy()


def _build():
    import concourse.bacc as bacc
    import concourse.mybir as mybir
    import concourse.tile as tile
    from concourse.ap import AP

    F32 = mybir.dt.float32
    F32R = mybir.dt.float32r
    BF16 = mybir.dt.bfloat16
    FP8 = mybir.dt.float8e4
    AF = mybir.ActivationFunctionType
    op = mybir.AluOpType
    PM = mybir.MatmulPerfMode

    nc = bacc.Bacc("TRN2", target_bir_lowering=False, debug=False)

    # The act-table chooser picks the FIRST set containing a function, so
    # Ln -> 'natural_log' and Exp -> 'exp_and_others' thrash table loads
    # (1283ns each) every nu refresh.  Hide ln/exp/square from the earlier
    # sets in the cached dict (order, and hence act_func_set_ids, stay
    # valid) so all three resolve to 'natural_log_exp_and_others' and the
    # load hoists out of the NNMF loop.
    from concourse.hw_specs import get_activation_tables
    tabs = get_activation_tables(nc.m.arch)
    combined = "natural_log_exp_and_others"
    if combined in tabs:
        hide = {AF.Ln, AF.Exp, AF.Square} & tabs[combined]
        for name, funcs in tabs.items():
            if name != combined:
                funcs -= hide

    x_ext = nc.declare_dram_parameter("x", [NB, DIM, H, W], F32R, isOutput=False)
    afwd_ext = nc.declare_dram_parameter("afwd", [NBLK, 128, 10, 128], FP8, isOutput=False)
    abwd_ext = nc.declare_dram_parameter("abwd", [NBLK, 128, 10, 128], FP8, isOutput=False)
    w1_ext = nc.declare_dram_parameter("w1", [128, NJ, NBLK, 128], FP8, isOutput=False)
    w2_ext = nc.declare_dram_parameter("w2", [128, NBLK, NJ // 2, 2, 128], FP8, isOutput=False)
    g1_ext = nc.declare_dram_parameter("g1", [NBLK, 128], F32, isOutput=False)
    b1_ext = nc.declare_dram_parameter("b1", [NBLK, 128], F32, isOutput=False)
    g2_ext = nc.declare_dram_parameter("g2", [NBLK, 128], F32, isOutput=False)
    b2_ext = nc.declare_dram_parameter("b2", [NBLK, 128], F32, isOutput=False)
    bf1_ext = nc.declare_dram_parameter("bf1", [NJ, 128], F32, isOutput=False)
    bf2_ext = nc.declare_dram_parameter("bf2", [NBLK, 128], F32, isOutput=False)
    out_ext = nc.declare_dram_parameter("out", [NB, DIM, H, W], F32, isOutput=True)

    # PSUM accumulation groups must not cross a 512-col (2KB) bank boundary
    RCH = [(0, 512), (512, RL - 512)]   # stats/MLP chunks (within [R0, R0+RL))
    HL = [(1, 0), (15, 512)]            # (first interior row, psum col)
    N14 = 14 * 28
    # tap t=(ky,kx): window start offset for output rows r0.. = (r0+ky-1)*PW+kx
    TOFF = [(ky - 1) * PW + (kx - 1) for ky in range(3) for kx in range(3)]

    with tile.TileContext(nc) as tc:
        with (
            tc.tile_pool(name="singles", bufs=1) as singles,
            tc.tile_pool(name="img", bufs=1) as pimg,
            tc.tile_pool(name="stats", bufs=3) as stats,
            tc.tile_pool(name="psA", bufs=3, space="PSUM") as psA,
            tc.tile_pool(name="psB", bufs=1, space="PSUM") as psB,
        ):
            # ---- weights / params resident in SBUF ----
            onesf = singles.tile([128, 128], F32)
            nc.vector.memset(onesf, 1.0)
            ones_r = singles.tile([128, 128], F32R)
            nc.vector.tensor_copy(ones_r, onesf)
            ones_b = singles.tile([128, 128], BF16)
            nc.vector.memset(ones_b, 1.0)
            # colsum-normalize weights: 1/HS folded in; exact 1.0 for last iter
            onesC = singles.tile([128, 128], BF16)
            nc.vector.memset(onesC, 1.0 / HS)
            wfwd = []
            wbwd = []
            for b in range(NBLK):
                wf = singles.tile([128, 10, 128], FP8, name=f"wfwd{b}", tag=f"wfwd{b}")
                nc.sync.dma_start(out=wf, in_=afwd_ext[b])
                wfwd.append(wf)
                wb = singles.tile([128, 10, 128], FP8, name=f"wbwd{b}", tag=f"wbwd{b}")
                nc.sync.dma_start(out=wb, in_=abwd_ext[b])
                wbwd.append(wb)
            w1s = singles.tile([128, NJ, NBLK, 128], FP8, name="w1s", tag="w1s")
            nc.sync.dma_start(out=w1s, in_=w1_ext[:, :, :, :])
            w2s = singles.tile([128, NBLK, NJ // 2, 2, 128], FP8, name="w2s",
                               tag="w2s")
            nc.sync.dma_start(out=w2s, in_=w2_ext[:, :, :, :, :])

            def load_param(ext, n, name):
                t = singles.tile([128, n], F32, name=name, tag=name)
                nc.sync.dma_start(out=t, in_=ext[:, :].rearrange("b p -> p b"))
                return t

            eps1_t = singles.tile([128, 1], F32, name="eps1_t", tag="eps1_t")
            nc.vector.memset(eps1_t, EPS)
            eps2_t = singles.tile([128, 1], F32, name="eps2_t", tag="eps2_t")
            nc.vector.memset(eps2_t, 1e-5)

            g1t = load_param(g1_ext, NBLK, "g1t")
            b1t = load_param(b1_ext, NBLK, "b1t")
            g2t = load_param(g2_ext, NBLK, "g2t")
            b2t = load_param(b2_ext, NBLK, "b2t")
            bf1t = load_param(bf1_ext, NJ, "bf1t")
            bf2t = load_param(bf2_ext, NBLK, "bf2t")

            def pad3(t, b):
                # [128, PLEN] slice of block b viewed as [128, 30, 30]
                return t[:, b, :].rearrange("p (r c) -> p r c", c=PW)

            def i4(t, b):
                # interior of block b as [128, 2, 14, 28]
                return (pad3(t, b)[:, 1:29, 1:29]
                        .rearrange("p (two r) c -> p two r c", two=2))

            def ps4(ps):
                # conv psum halves as [128, 2, 14, 28]
                return (ps.rearrange("p (two x) -> p two x", two=2)
                        [:, :, 0:N14]
                        .rearrange("p two (r c) -> p two r c", c=28))

            def ps3(ps):
                # conv psum halves as [128, 2, 392]
                return ps.rearrange("p (two x) -> p two x", two=2)[:, :, 0:N14]

            def cp4(t, b):
                # compact 784-col block of t as [128, 2, 14, 28]
                return t[:, b, 0:784].rearrange("p (two r c) -> p two r c",
                                                two=2, c=28)

            def conv_block(dst_ps, wtile, src, b):
                """3x3 grouped conv of block b via 5 fp8 DoubleRow pair
                matmuls per half: pairs (0,1)(2,3)(4,5)(6,7)(8,zero)."""
                base = src.offset + b * PLEN
                pstr = src.ap[0]
                for (r0, c0) in HL:
                    for k in range(5):
                        t0 = 2 * k
                        o0 = r0 * PW + 1 + TOFF[t0]
                        if k < 4:
                            dlt = TOFF[t0 + 1] - TOFF[t0]
                        else:
                            dlt = 0  # zero-weight dummy reuses tap8 window
                        rhs = AP(src.tensor, base + o0,
                                 [pstr, [dlt, 2], [PW, 14], [1, 28]])
                        nc.tensor.matmul(
                            out=dst_ps[:, c0:c0 + N14],
                            lhsT=wtile[:, t0:t0 + 2, :],
                            rhs=rhs,
                            start=(k == 0), stop=(k == 4),
                            perf_mode=PM.DoubleRow,
                        )

            def layernorm(src, dst_slice_fn, eps, gt, bt, post):
                """Channel LN over the 3 partition blocks of `src`
                [128,NBLK,PLEN] (f32r) on range [R0, R0+RL)."""
                s1 = psB.tile([128, 1024], F32, tag="sum")
                for (c0, cn) in RCH:
                    for b in range(NBLK):
                        nc.tensor.matmul(
                            out=s1[:, c0:c0 + cn],
                            lhsT=ones_r,
                            rhs=src[:, b, R0 + c0: R0 + c0 + cn],
                            start=(b == 0),
                            stop=(b == NBLK - 1),
                        )
                sqs = []
                for b in range(NBLK):
                    sq = stats.tile([128, RL], BF16, tag="sq", bufs=3)
                    nc.scalar.activation(
                        out=sq, in_=src[:, b, R0:R0 + RL].bitcast(F32), func=AF.Square
                    )
                    sqs.append(sq)
                s2 = psB.tile([128, 1024], F32, tag="sum")
                for (c0, cn) in RCH:
                    for b in range(NBLK):
                        nc.tensor.matmul(
                            out=s2[:, c0:c0 + cn],
                            lhsT=ones_b,
                            rhs=sqs[b][:, c0:c0 + cn],
                            start=(b == 0),
                            stop=(b == NBLK - 1),
                        )
                m = stats.tile([128, RL], F32, tag="mstat", bufs=4)
                nc.vector.tensor_scalar_mul(m, s1[:, 0:RL], 1.0 / DIM)
                t2 = stats.tile([128, RL], F32, tag="mstat", bufs=4)
                nc.vector.tensor_scalar_mul(t2, s2[:, 0:RL], 1.0 / DIM)
                msq = stats.tile([128, RL], F32, tag="mstat", bufs=4)
                nc.scalar.activation(out=msq, in_=m, func=AF.Square)
                v = stats.tile([128, RL], F32, tag="mstat", bufs=4)
                nc.vector.tensor_sub(v, t2, msq)
                # rstd = exp(-0.5*ln(v+eps)): stays inside the ln/exp
                # activation-table set (no Sqrt table load, no DVE recip)
                lv = stats.tile([128, RL], F32, tag="mstat", bufs=4)
                nc.scalar.activation(out=lv, in_=v, func=AF.Ln, bias=eps)
                rstd = stats.tile([128, RL], F32, tag="mstat", bufs=4)
                nc.scalar.activation(out=rstd, in_=lv, func=AF.Exp, scale=-0.5)

                if post == "ln1":
                    # produce lnx = ln(SCL*z0) - ln(colsum z0) directly
                    lnx = dst_slice_fn
                    z0s = []
                    for b in range(NBLK):
                        d = stats.tile([128, RL], F32, tag="dtmp", bufs=1)
                        nc.vector.tensor_sub(d, src[:, b, R0:R0 + RL].bitcast(F32), m)
                        xn = stats.tile([128, RL], F32, tag="dtmp2", bufs=1)
                        nc.vector.tensor_mul(xn, d, rstd)
                        nc.vector.tensor_scalar(
                            xn, xn, gt[:, b:b + 1], bt[:, b:b + 1], op.mult, op.add
                        )
                        z0 = stats.tile([128, RL], BF16, tag="z0", bufs=3)
                        nc.vector.tensor_scalar_max(z0, xn, EPS)
                        z0s.append(z0)
                    s0ps = psB.tile([128, 1024], F32, tag="sum")
                    for (c0, cn) in RCH:
                        for b in range(NBLK):
                            nc.tensor.matmul(
                                out=s0ps[:, c0:c0 + cn],
                                lhsT=ones_b,
                                rhs=z0s[b][:, c0:c0 + cn],
                                start=(b == 0),
                                stop=(b == NBLK - 1),
                            )
                    for b in range(NBLK):
                        zi = AP(z0s[b].tensor, z0s[b].offset + 1,
                                [z0s[b].ap[0], [420, 2], [PW, 14], [1, 28]])
                        nc.scalar.activation(
                            out=lnx(b), in_=zi, func=AF.Ln, scale=SCL,
                        )
                    # interior [p,2,14,28] view of the colsum psum
                    si = AP(s0ps.tensor, s0ps.offset + 1,
                            [s0ps.ap[0], [420, 2], [PW, 14], [1, 28]])
                    lnS0 = stats.tile([128, 784], BF16, tag="rcp", bufs=2)
                    nc.scalar.activation(
                        out=lnS0.rearrange("p (two r c) -> p two r c",
                                           two=2, c=28),
                        in_=si, func=AF.Ln,
                    )
                    return lnS0
                else:
                    for b in range(NBLK):
                        d = stats.tile([128, RL], F32, tag="dtmp", bufs=1)
                        nc.vector.tensor_sub(d, src[:, b, R0:R0 + RL].bitcast(F32), m)
                        xn = stats.tile([128, RL], F32, tag="dtmp2", bufs=1)
                        nc.vector.tensor_mul(xn, d, rstd)
                        nc.vector.tensor_scalar(
                            dst_slice_fn(b), xn, gt[:, b:b + 1], bt[:, b:b + 1],
                            op.mult, op.add,
                        )

            # ================= per image (pairs interleaved) =================
            def setup_image(img):
                xpad = pimg.tile([128, NBLK, PLEN], F32R, tag="xpad", bufs=4,
                                 name=f"xpad{img}")
                hT = pimg.tile([128, NBLK, PLEN], FP8, tag="h", bufs=4,
                               name=f"h{img}")
                nuT = pimg.tile([128, NBLK, PLEN], FP8, tag="nu", bufs=4,
                                name=f"nu{img}")
                uT = pimg.tile([128, NBLK, PLEN], BF16, tag="u", bufs=4,
                               name=f"u{img}")
                # 840 cols so the tail can reuse this slot for the LN2 out
                lnx = pimg.tile([128, NBLK, RL], BF16, tag="lnx", bufs=4,
                                name=f"lnx{img}")
                nc.gpsimd.memset(hT, 0.0)
                nc.gpsimd.memset(nuT, 0.0)
                for b in range(NBLK):
                    nc.sync.dma_start(
                        out=pad3(xpad, b)[:, 1:29, 1:29],
                        in_=x_ext[img, b * 128:(b + 1) * 128, :, :],
                    )
                    nc.gpsimd.memset(pad3(hT, b)[:, 1:29, 1:29], HS / DIM)
                # LN1 emits lnx = ln(SCL*z0) per block; subtract ln(colsum)
                lnS0 = layernorm(
                    xpad, lambda b: cp4(lnx, b), eps1_t, g1t, b1t, "ln1"
                )
                nc.vector.tensor_sub(
                    lnx[:, :, 0:784],
                    lnx[:, :, 0:784],
                    AP(lnS0.tensor, lnS0.offset,
                       [lnS0.ap[0], [0, NBLK], [1, 784]]),
                )
                return xpad, hT, nuT, uT, lnx

            def lnr4(uT, b):
                # lnr scratch lives in uT[:, b, 0:784] (dead between h-mul
                # and the next u-mul write)
                return uT[:, b, 0:784].rearrange("p (two r c) -> p two r c",
                                                 two=2, c=28)

            def nu_phase_recon(ts):
                xpad, hT, nuT, uT, lnx = ts
                for b in range(NBLK):
                    ps = psA.tile([128, 1024], F32, tag="conv")
                    conv_block(ps, wbwd[b], hT, b)
                    nc.scalar.activation(
                        out=lnr4(uT, b).rearrange("p t r c -> p t (r c)"),
                        in_=ps3(ps), func=AF.Ln,
                    )

            def nu_phase_subexp(ts):
                xpad, hT, nuT, uT, lnx = ts
                nc.vector.tensor_sub(
                    uT[:, :, 0:784],
                    lnx[:, :, 0:784],
                    uT[:, :, 0:784],
                )
                for b in range(NBLK):
                    nc.scalar.activation(
                        out=i4(nuT, b), in_=lnr4(uT, b), func=AF.Exp,
                    )

            def u_phase(ts, norm_it):
                xpad, hT, nuT, uT, lnx = ts
                for b in range(NBLK):
                    ps = psA.tile([128, 1024], F32, tag="conv")
                    conv_block(ps, wfwd[b], nuT, b)
                    if norm_it:
                        nc.vector.tensor_mul(i4(uT, b), i4(hT, b), ps4(ps))
                    else:
                        # unnormalized iteration: h <- h * conv/WS in place
                        nc.vector.scalar_tensor_tensor(
                            out=i4(hT, b), in0=ps4(ps), scalar=1.0 / WS,
                            in1=i4(hT, b), op0=op.mult, op1=op.mult,
                        )

            def norm_phase(ts, it):
                xpad, hT, nuT, uT, lnx = ts
                last = (it == ITERS - 1)
                ss = psB.tile([128, 1024], F32, tag="sum")
                ones_t = ones_b if last else onesC
                for (r0, c0) in HL:
                    for b in range(NBLK):
                        nc.tensor.matmul(
                            out=ss[:, c0:c0 + N14],
                            lhsT=ones_t,
                            rhs=pad3(uT, b)[:, r0:r0 + 14, 1:29],
                            start=(b == 0),
                            stop=(b == NBLK - 1),
                        )
                if last:
                    sinv = stats.tile([128, 784], F32, tag="sinv", bufs=2)
                    nc.vector.reciprocal_approx_fast(
                        out=sinv.rearrange("p (two x) -> p two x", two=2),
                        in_=ps3(ss),
                    )
                else:
                    # sinv = exp(-ln(colsum)) on ACT: frees DVE, bf16 out
                    # makes the h-mul an all-bf16 2x op
                    lcs = stats.tile([128, 784], BF16, tag="lcs", bufs=2)
                    nc.scalar.activation(
                        out=lcs.rearrange("p (two x) -> p two x", two=2),
                        in_=ps3(ss), func=AF.Ln,
                    )
                    sinv = stats.tile([128, 784], BF16, tag="sinv8", bufs=2)
                    nc.scalar.activation(
                        out=sinv.rearrange("p (two x) -> p two x", two=2),
                        in_=lcs.rearrange("p (two x) -> p two x", two=2),
                        func=AF.Exp, scale=-1.0,
                    )
                s4 = sinv.rearrange("p (two r c) -> p two r c", two=2, c=28)
                # last iter: write true h in-place into uT (bf16)
                dstT = uT if last else hT
                for b in range(NBLK):
                    eng = nc.gpsimd if (b >= NBLK - POOL_HMUL and not last) \
                        else nc.vector
                    eng.tensor_mul(i4(dstT, b), i4(uT, b), s4)

            def tail_ln(img, ts):
                xpad, hT, nuT, uT, lnx = ts
                # residual in place: xpad <- x + h  (final h lives in uT)
                for b in range(NBLK):
                    nc.vector.tensor_add(
                        xpad[:, b, R0:R0 + RL],
                        xpad[:, b, R0:R0 + RL].bitcast(F32),
                        uT[:, b, R0:R0 + RL],
                    )
                # LN2 -> bf16 (feeds bf16 fc1)
                ln2o = pimg.tile([128, NBLK, RL], FP8, tag="ln2o", bufs=4,
                                 name=f"ln2o{img}")
                layernorm(
                    xpad, lambda b: ln2o[:, b, :], eps2_t, g2t, b2t, "ln2"
                )
                return ln2o

            def tail_mlp(img, ts, ln2o):
                xpad, hT, nuT, uT, lnx = ts
                # fp8 DoubleRow MLP: fc1 pairs the first two k-blocks
                # (+1 plain fp8 matmul), fc2 pairs hid j-slices; weight
                # scales W1S/W2S are undone via activation scale inputs.
                obufs = [psA.tile([128, 1024], F32, tag="conv",
                                  name=f"ob{img}_{cb}")
                         for cb in range(NBLK)]
                for k in range(NJ // 2):
                    hidp = pimg.tile([128, 2, RL], FP8, tag="hid", bufs=2,
                                     name=f"hid{img}_{k}")
                    for jj in range(2):
                        j = 2 * k + jj
                        hp = psB.tile([128, 1024], F32, tag="sum")
                        for (c0, cn) in RCH:
                            nc.tensor.matmul(
                                out=hp[:, c0:c0 + cn],
                                lhsT=w1s[:, j, 0:2, :],
                                rhs=ln2o[:, 0:2, c0:c0 + cn],
                                start=True, stop=False,
                                perf_mode=PM.DoubleRow,
                            )
                            nc.tensor.matmul(
                                out=hp[:, c0:c0 + cn],
                                lhsT=w1s[:, j, 2, :],
                                rhs=ln2o[:, 2, c0:c0 + cn],
                                start=False, stop=True,
                            )
                        nc.scalar.activation(
                            out=hidp[:, jj, :], in_=hp[:, 0:RL], func=AF.Gelu,
                            bias=bf1t[:, j:j + 1], scale=1.0 / W1S,
                        )
                    for cb in range(NBLK):
                        for (c0, cn) in RCH:
                            nc.tensor.matmul(
                                out=obufs[cb][:, c0:c0 + cn],
                                lhsT=w2s[:, cb, k, :, :],
                                rhs=hidp[:, 0:2, c0:c0 + cn],
                                start=(k == 0), stop=(k == NJ // 2 - 1),
                                perf_mode=PM.DoubleRow,
                            )
                for cb in range(NBLK):
                    mo = stats.tile([128, RL], BF16, tag="mlpout", bufs=2)
                    nc.scalar.activation(
                        out=mo, in_=obufs[cb][:, 0:RL], func=AF.Identity,
                        bias=bf2t[:, cb:cb + 1], scale=1.0 / W2S,
                    )
                    # final output written in-place into xpad (f32r rounding)
                    nc.vector.tensor_add(
                        xpad[:, cb, R0:R0 + RL],
                        xpad[:, cb, R0:R0 + RL].bitcast(F32),
                        mo,
                    )
                for b in range(NBLK):
                    nc.sync.dma_start(
                        out=out_ext[img, b * 128:(b + 1) * 128, :, :],
                        in_=pad3(xpad.bitcast(F32), b)[:, 1:29, 1:29],
                    )

            imgs = list(range(NB))
            tsets = {img: setup_image(img) for img in imgs}
            for it in range(ITERS):
                # phase-level interleave: later images' PE work overlaps
                # earlier images' ACT/DVE chains
                if it % NU_EVERY == 0:
                    # skewed: img k's sub/exp slot between img k+1's and
                    # img k+2's recon so ACT work never gates PE
                    prev = None
                    for img in imgs:
                        nu_phase_recon(tsets[img])
                        if prev is not None:
                            nu_phase_subexp(tsets[prev])
                        prev = img
                    nu_phase_subexp(tsets[prev])
                norm_it = (it % 2 == 1) or (it == ITERS - 1)
                for img in imgs:
                    u_phase(tsets[img], norm_it)
                if norm_it:
                    for img in imgs:
                        norm_phase(tsets[img], it)
            lns = {img: tail_ln(img, tsets[img]) for img in imgs}
            for img in imgs:
                tail_mlp(img, tsets[img], lns[img])

    nc.compile()
    return nc


def _prep_weights(Wc, g1, b1, g2, b2, w_fc1, b_fc1, w_fc2, b_fc2):
    import ml_dtypes

    FP8NP = ml_dtypes.float8_e4m3
    wp = np.abs(np.asarray(Wc, np.float32))
    wp = wp / np.maximum(wp.sum(axis=(1, 2, 3), keepdims=True), EPS)
    wp4 = wp.reshape(NBLK, 2, 64, 64, 3, 3)  # [b, gi, co, ci, ky, kx]
    afwd = np.zeros((NBLK, 128, 10, 128), np.float32)
    abwd = np.zeros((NBLK, 128, 10, 128), np.float32)
    for b in range(NBLK):
        for gi in range(2):
            blk = wp4[b, gi] * WS
            afwd[b, gi * 64:(gi + 1) * 64, 0:9, gi * 64:(gi + 1) * 64] = (
                blk.transpose(1, 2, 3, 0).reshape(64, 9, 64)
            )
            abwd[b, gi * 64:(gi + 1) * 64, 0:9, gi * 64:(gi + 1) * 64] = (
                blk[:, :, ::-1, ::-1].transpose(0, 2, 3, 1).reshape(64, 9, 64)
            )
    # fc1: [p, j, kb, m] = w_fc1[kb*128+p, j*128+m] * W1S
    w1 = (np.asarray(w_fc1, np.float32).reshape(NBLK, 128, NJ, 128)
          .transpose(1, 2, 0, 3) * W1S)
    # fc2: [p, cb, k, i, m] = w_fc2[(2k+i)*128+p, cb*128+m] * W2S
    w2 = (np.asarray(w_fc2, np.float32).reshape(NJ // 2, 2, 128, NBLK, 128)
          .transpose(2, 3, 0, 1, 4) * W2S)
    return {
        "afwd": afwd.astype(FP8NP),
        "abwd": abwd.astype(FP8NP),
        "w1": w1.astype(FP8NP),
        "w2": w2.astype(FP8NP),
        "g1": np.asarray(g1, np.float32).reshape(NBLK, 128),
        "b1": np.asarray(b1, np.float32).reshape(NBLK, 128),
        "g2": np.asarray(g2, np.float32).reshape(NBLK, 128),
        "b2": np.asarray(b2, np.float32).reshape(NBLK, 128),
        "bf1": np.asarray(b_fc1, np.float32).reshape(NJ, 128),
        "bf2": np.asarray(b_fc2, np.float32).reshape(NBLK, 128),
    }


_last_result = None


def kernel(x, g1, b1, Wc, g2, b2, w_fc1, b_fc1, w_fc2, b_fc2):
    global _last_result
    # The kernel needs the axon NeuronCore jax backend; a leftover
    # JAX_PLATFORMS=cpu pin (used for running the jax reference) would hide
    # the devices.  Best-effort: clear it before jax initializes.
    if os.environ.get("JAX_PLATFORMS", "").strip().lower() == "cpu":
        del os.environ["JAX_PLATFORMS"]
    from concourse.bass_utils import run_bass_kernel_spmd

    if "nc" not in _cache:
        _cache["nc"] = _build()
    nc = _cache["nc"]

    shared = _prep_weights(Wc, g1, b1, g2, b2, w_fc1, b_fc1, w_fc2, b_fc2)
    x = np.asarray(x, np.float32)
    assert x.shape == (NB * NCORES, DIM, H, W), x.shape
    in_maps = []
    for c in range(NCORES):
        m = dict(shared)
        m["x"] = np.ascontiguousarray(x[c * NB:(c + 1) * NB])
        in_maps.append(m)

    r = run_bass_kernel_spmd(
        nc, in_maps, list(range(NCORES)),
        trace=bool(os.environ.get("K_TRACE")),
    )
    _last_result = r
    out = np.concatenate(
        [r.results[c]["out"] for c in range(NCORES)], axis=0
    ).astype(np.float32)
    return out
